# revision 12
# baseline (speedup 1.0000x reference)
"""Trainium2 Bass kernel for nn_Centerdist (segment variance loss).

Math: for each id k in [0, 1000):
    loss_k = sum_{i: id_i=k} ||x_i - mean_k||^2 / n_k
           = (sumsq_k - ||sums_k||^2 / n_k) / n_k
    loss = sum_k loss_k / n_uniq

So a single pass computing per-id (sums[K, D], sumsq[K]) suffices; counts
come from a host-side bincount over the small ids array.  The device does
the O(N*D) work: for every 128-row tile, build a one-hot [128, K] matrix
(ids vs iota compare on DVE), compute per-row ||x||^2 (fused
tensor_tensor_reduce), and matmul-accumulate one_hot.T @ [X | rownormsq]
into persistent PSUM accumulators (8 banks = 8 chunks of 128 ids).

Sharding: data-parallel over N across 8 NeuronCores; per-core partial
[1024, 257] outputs are summed on host (the all-reduce is 8 MB total, so
host summation replaces an on-chip collective).
"""

import numpy as np

from concourse import bacc, bass, bass_utils, mybir, tile

F32 = mybir.dt.float32
F16 = mybir.dt.float16

N_FULL = 262144
D = 256
K = 1024  # padded id range (real ids < 1000)
P = 128
N_CORES = 8
NS = N_FULL // N_CORES  # 32768 rows per core
KC = K // P  # 8 id chunks of 128
RW = D + 1  # rhs width: [X | rownormsq]
LOAD_T = 4  # tiles per cast-DMA load (amortizes SWDGE fixed cost)


def build_program(tiles: int, reps: int = 1):
    """Build the per-core Bass program processing `tiles` 128-row tiles.

    reps>1 repeats the whole pass (for slope-based HW timing); the output
    is identical since each rep restarts the PSUM accumulation groups.
    """
    nc = bacc.Bacc(
        "TRN2",
        target_bir_lowering=False,
        debug=False,
        num_devices=N_CORES,
    )
    ns = tiles * P
    load_t = min(LOAD_T, tiles)
    assert tiles % load_t == 0
    x_d = nc.dram_tensor("x", [ns, D], F32, kind="ExternalInput")
    idst_d = nc.dram_tensor("idst", [P, tiles], F32, kind="ExternalInput")
    iota_d = nc.dram_tensor("iota", [P, K], F16, kind="ExternalInput")
    out_d = nc.dram_tensor("out", [KC, P, RW], F32, kind="ExternalOutput")

    with tile.TileContext(nc) as tc:
        with (
            tc.tile_pool(name="const", bufs=1) as cpool,
            tc.tile_pool(name="xp", bufs=3) as xpool,
            tc.tile_pool(name="sqp", bufs=2) as sqpool,
            tc.tile_pool(name="rqp", bufs=2) as rqpool,
            tc.tile_pool(name="ohp", bufs=3) as ohpool,
            tc.tile_pool(name="psp", bufs=1, space="PSUM") as pspool,
            tc.tile_pool(name="evp", bufs=2) as evpool,
        ):
            iota_t = cpool.tile([P, K], F16, tag="iota")
            nc.sync.dma_start(iota_t[:], iota_d.ap())
            idst_t = cpool.tile([P, tiles], F32, tag="idst")
            nc.sync.dma_start(idst_t[:], idst_d.ap())

            psums = [
                pspool.tile([P, 512], F32, name=f"ps{c}", tag=f"ps{c}")
                for c in range(KC)
            ]

            # [group, p, t, d] view of row-major x for multi-tile cast loads
            x_g = x_d.ap().rearrange("(g t p) d -> g p t d", p=P, t=load_t)
            for rep in range(reps):
              for tg in range(tiles // load_t):
                # fp32 -> fp16 cast happens inside the SWDGE DMA
                xt4 = xpool.tile([P, load_t, D + 2], F16, name="xt4", tag="xt4")
                nc.gpsimd.dma_start(xt4[:, :, 0:D], x_g[tg])

                # squares for the whole load on the (otherwise idle) ACT engine
                sq4 = sqpool.tile([P, load_t, D], F16, name="sq4", tag="sq4")
                nc.scalar.activation(
                    sq4[:], xt4[:, :, 0:D], mybir.ActivationFunctionType.Square
                )

                for tt in range(load_t):
                    t = tg * load_t + tt
                    # rownormsq: fp32 row-sum of squares, cast into rhs col D
                    rq = rqpool.tile([P, 1], F32, name="rq", tag="rq")
                    nc.vector.tensor_reduce(
                        out=rq[:],
                        in_=sq4[:, tt, :],
                        axis=mybir.AxisListType.X,
                        op=mybir.AluOpType.add,
                    )
                    nc.vector.tensor_copy(xt4[:, tt, D : D + 1], rq[:])

                    oh = ohpool.tile([P, K], F16, name="oh", tag="oh")
                    nc.vector.tensor_scalar(
                        out=oh[:],
                        in0=iota_t[:],
                        scalar1=idst_t[:, t : t + 1],
                        scalar2=None,
                        op0=mybir.AluOpType.is_equal,
                    )

                    for c in range(KC):
                        nc.tensor.matmul(
                            psums[c][:, 0:RW],
                            oh[:, c * P : (c + 1) * P],
                            xt4[:, tt, 0:RW],
                            start=(t == 0),
                            stop=(t == tiles - 1),
                        )

            out_ap = out_d.ap()
            for c in range(KC):
                ev = evpool.tile([P, RW], F32, name="ev", tag="ev")
                nc.vector.tensor_copy(ev[:], psums[c][:, 0:RW])
                nc.sync.dma_start(out_ap[c], ev[:])

    nc.compile()
    return nc


_PROGRAM_CACHE: dict = {}


def _get_program(tiles: int, reps: int = 1):
    key = (tiles, reps)
    if key not in _PROGRAM_CACHE:
        _PROGRAM_CACHE[key] = build_program(tiles, reps)
    return _PROGRAM_CACHE[key]


def make_in_maps(reid_feat: np.ndarray, ids: np.ndarray):
    """Shard inputs for the 8 cores: row-shard x, transposed fp32 ids."""
    x = np.ascontiguousarray(np.asarray(reid_feat, dtype=np.float32))
    ids_np = np.asarray(ids).astype(np.int64)
    n = x.shape[0]
    valid = ids_np >= 0
    # invalid ids -> -1.0 never matches the iota row, so those rows drop out
    seg_f = np.where(valid, ids_np, -1).astype(np.float32)

    ns = n // N_CORES
    tiles = ns // P
    xs = x.reshape(N_CORES, ns, D)
    # idst[c][p, t] = seg_f[c*ns + t*128 + p]; ids < 1024 are exact in fp16
    idst = (
        seg_f.reshape(N_CORES, tiles, P).transpose(0, 2, 1).copy()
    )
    iota = np.broadcast_to(np.arange(K, dtype=np.float16), (P, K)).copy()
    in_maps = [
        {"x": xs[c], "idst": idst[c], "iota": iota} for c in range(N_CORES)
    ]
    return in_maps, tiles, valid


def finalize(parts: np.ndarray, ids: np.ndarray, valid: np.ndarray) -> np.ndarray:
    """Combine per-core partials [cores, KC, P, RW] into the scalar loss."""
    agg = parts.astype(np.float64).sum(axis=0).reshape(K, RW)
    sums = agg[:1000, :D]
    sumsq = agg[:1000, D]
    ids_np = np.asarray(ids).astype(np.int64)
    counts = np.bincount(ids_np[valid], minlength=1000)[:1000].astype(np.float64)
    safe_n = np.maximum(counts, 1.0)
    sq_per_id = sumsq - (sums * sums).sum(axis=1) / safe_n
    per_id_loss = np.where(counts > 0, sq_per_id / safe_n, 0.0)
    n_uniq = float((counts > 0).sum()) + (1.0 if (~valid).any() else 0.0)
    return np.array(per_id_loss.sum() / n_uniq, dtype=np.float32)


def run_device(reid_feat, ids, trace: bool = False):
    in_maps, tiles, valid = make_in_maps(reid_feat, ids)
    nc = _get_program(tiles)
    res = bass_utils.run_bass_kernel_spmd(
        nc, in_maps, core_ids=list(range(N_CORES)), trace=trace
    )
    parts = np.stack([res.results[c]["out"] for c in range(N_CORES)])
    return parts, valid, res


class DeviceRunner:
    """Persistent jitted SPMD executor (mirrors bass2jax.run_bass_via_pjrt)
    so a program can be executed many times for timing without re-tracing."""

    def __init__(self, nc, in_maps):
        import jax
        from jax.sharding import Mesh, PartitionSpec
        from jax.experimental.shard_map import shard_map
        from concourse import bass2jax, mybir as mb

        bass2jax.install_neuronx_cc_hook()
        partition_name = (
            nc.partition_id_tensor.name if nc.partition_id_tensor else None
        )
        in_names, out_names, out_avals, zero_outs = [], [], [], []
        for alloc in nc.m.functions[0].allocations:
            if not isinstance(alloc, mb.MemoryLocationSet):
                continue
            name = alloc.memorylocations[0].name
            if alloc.kind == "ExternalInput":
                if name != partition_name:
                    in_names.append(name)
            elif alloc.kind == "ExternalOutput":
                shape = tuple(alloc.tensor_shape)
                npdt = np.dtype(mb.dt.np(alloc.dtype))
                out_names.append(name)
                out_avals.append(jax.core.ShapedArray(shape, npdt))
                zero_outs.append(np.zeros(shape, npdt))
        self.out_names = out_names
        n_params = len(in_names)
        n_outs = len(out_avals)
        all_names = list(in_names) + list(out_names)
        if partition_name is not None:
            all_names.append(partition_name)

        def _body(*args):
            operands = list(args)
            if partition_name is not None:
                operands.append(bass2jax.partition_id_tensor())
            outs = bass2jax._bass_exec_p.bind(
                *operands,
                out_avals=tuple(out_avals),
                in_names=tuple(all_names),
                out_names=tuple(out_names),
                lowering_input_output_aliases=(),
                sim_require_finite=True,
                sim_require_nnan=True,
                nc=nc,
            )
            return tuple(outs)

        devices = jax.devices()[:N_CORES]
        mesh = Mesh(np.asarray(devices), ("core",))
        in_specs = (PartitionSpec("core"),) * (n_params + n_outs)
        out_specs = (PartitionSpec("core"),) * n_outs
        self._fn = jax.jit(
            shard_map(
                _body,
                mesh=mesh,
                in_specs=in_specs,
                out_specs=out_specs,
                check_rep=False,
            ),
            keep_unused=True,
        )
        self._jax = jax
        concat_in = [
            np.concatenate([np.asarray(in_maps[c][nm]) for c in range(N_CORES)], axis=0)
            for nm in in_names
        ]
        concat_zeros = [
            np.zeros((N_CORES * z.shape[0], *z.shape[1:]), z.dtype) for z in zero_outs
        ]
        self._args = [jax.device_put(a) for a in concat_in + concat_zeros]
        self.out_shapes = [a.shape for a in out_avals]

    def run_once(self):
        outs = self._fn(*self._args)
        self._jax.block_until_ready(outs)
        return outs

    def results(self):
        outs = self.run_once()
        return [
            {
                nm: np.asarray(outs[i]).reshape(N_CORES, *self.out_shapes[i])[c]
                for i, nm in enumerate(self.out_names)
            }
            for c in range(N_CORES)
        ]

    def time_exec(self, iters: int = 20, warmup: int = 3):
        import time as _time

        for _ in range(warmup):
            self.run_once()
        times = []
        for _ in range(iters):
            t0 = _time.perf_counter()
            self.run_once()
            times.append(_time.perf_counter() - t0)
        return float(np.median(times)), times


def kernel(reid_feat, ids) -> np.ndarray:
    parts, valid, _ = run_device(reid_feat, ids)
    return finalize(parts, np.asarray(ids), valid)
